# revision 13
# baseline (speedup 1.0000x reference)
"""GCN autoencoder on 8 trn2 cores — 4 launches, no collectives.

Interleaved dest-row sharding (core c owns global rows g with g%8==c,
local index g//8). Host does the gathers between launches (uncounted).

  l1: own-rows (x @ W1)^T, W1-stationary             -> h1T [32, M_PAD] bf16
  l2: compact SpMM (A @ h1) + relu + @W2             -> tT  [16, M_PAD] bf16
  l3: compact SpMM (A @ t)                           -> zT  [16, M_PAD] bf16
  l4: z @ z^T upper-tri n-tiles, 4-way tile_position quads, contiguous
      per-t bf16 writes; host mirrors the triangle.

The SpMM avoids the dense 10240x10240 A: for each 128-dest-row tile only
~3.3k of the 10240 source nodes have an edge into it, so the host builds a
per-tile compacted B [NMAX,128] (B[s,d] = sum of edge weights s->d) plus the
matching gathered h rows, packed into one fp8 tensor bh [128, NTILE*S, 128+h].
u^T_tile = (h_c)^T @ B via DoubleRow fp8 matmuls.
"""

import sys

sys.path.insert(0, "/opt/trn_rl_repo")

import numpy as np
import ml_dtypes

import concourse.bacc as bacc
import concourse.mybir as mybir
import concourse.tile as tile
from concourse.bass_utils import run_bass_kernel_spmd

BF16 = ml_dtypes.bfloat16
FP8 = ml_dtypes.float8_e4m3

NC = 8
N = 10000
F = 512
H1 = 32
H2 = 16
M_SH = N // NC  # 1250 rows per core
M_PAD = 1280
NT = M_PAD // 128  # 10 dest tiles per core
KX = F // 128

_cache = {}
_last_maps = {}
_last_nc = {}
_launch_order = ["l1", "l2", "l3", "l4"]


def _new_nc():
    return bacc.Bacc("TRN2", target_bir_lowering=False, debug=False, num_devices=NC)


def _build_l1():
    """h1T = (x_own @ W1)^T, W1-slice stationary, x streaming."""
    nc = _new_nc()
    xt = nc.dram_tensor("xt", [128, KX, M_PAD], mybir.dt.bfloat16, kind="ExternalInput")
    w1 = nc.dram_tensor("w1", [128, KX, H1], mybir.dt.bfloat16, kind="ExternalInput")
    out = nc.dram_tensor("h1t", [H1, M_PAD], mybir.dt.bfloat16, kind="ExternalOutput")
    CH = [(0, 512), (512, 512), (1024, 256)]
    with tile.TileContext(nc) as tc:
        with (
            tc.tile_pool(name="sb", bufs=1) as sb,
            tc.tile_pool(name="pss", bufs=1, space="PSUM") as pss,
        ):
            xsb = sb.tile([128, KX, M_PAD], mybir.dt.bfloat16)
            w1sb = sb.tile([128, KX, H1], mybir.dt.bfloat16)
            h1sb = sb.tile([H1, M_PAD], mybir.dt.bfloat16)
            nc.sync.dma_start(out=w1sb[:], in_=w1[:])
            nc.sync.dma_start(out=xsb[:], in_=xt[:])
            pss_t = [pss.tile([H1, nn], mybir.dt.float32, tag=f"c{i}", name=f"c{i}")
                     for i, (n0, nn) in enumerate(CH)]
            for kx in range(KX):
                for i, (n0, nn) in enumerate(CH):
                    nc.tensor.matmul(
                        out=pss_t[i][:], lhsT=w1sb[:, kx, :],
                        rhs=xsb[:, kx, n0:n0 + nn],
                        start=(kx == 0), stop=(kx == KX - 1),
                    )
            for i, (n0, nn) in enumerate(CH):
                nc.vector.tensor_copy(out=h1sb[:, n0:n0 + nn], in_=pss_t[i][:])
            nc.sync.dma_start(out=out[:], in_=h1sb[:])
    nc.compile()
    return nc


def _build_spmm(S, h_dim, relu_w2):
    """Compact A-contraction: per dest tile, S k-slices of 128 compacted
    sources; bh[:, tile*S+s, :128] = B rows, [128:128+h_dim] = gathered h.
    relu_w2: apply relu then @W2 (l2) else plain copy-out (l3)."""
    nc = _new_nc()
    W = 128 + h_dim
    KB = NT * S
    bh = nc.dram_tensor("bh", [128, KB, W], mybir.dt.float8e4, kind="ExternalInput")
    if relu_w2:
        w2 = nc.dram_tensor("w2", [H1, H2], mybir.dt.float32, kind="ExternalInput")
        out = nc.dram_tensor("tt", [H2, M_PAD], mybir.dt.bfloat16, kind="ExternalOutput")
    else:
        out = nc.dram_tensor("zt", [h_dim, M_PAD], mybir.dt.bfloat16, kind="ExternalOutput")
    DRM = mybir.MatmulPerfMode.DoubleRow
    NCHUNK = 5
    with tile.TileContext(nc) as tc:
        with (
            tc.tile_pool(name="sb", bufs=1) as sb,
            tc.tile_pool(name="psu", bufs=4, space="PSUM") as psu,
            tc.tile_pool(name="psw", bufs=1, space="PSUM") as psw,
        ):
            bhsb = sb.tile([128, KB, W], mybir.dt.float8e4)
            per = KB // NCHUNK
            for b in range(NCHUNK):
                dq = nc.sync if b % 2 == 0 else nc.scalar
                dq.dma_start(out=bhsb[:, b * per:(b + 1) * per, :],
                             in_=bh[:, b * per:(b + 1) * per, :])
            if relu_w2:
                w2sb = sb.tile([H1, H2], mybir.dt.float32)
                nc.sync.dma_start(out=w2sb[:], in_=w2[:])
                uT = sb.tile([H1, M_PAD], mybir.dt.float32)
            osb = sb.tile([h_dim if not relu_w2 else H2, M_PAD], mybir.dt.bfloat16)
            for t in range(NT):
                acc = psu.tile([h_dim, 128], mybir.dt.float32, tag="u")
                for j in range(S // 2):
                    k = t * S + 2 * j
                    nc.tensor.matmul(
                        out=acc[:], lhsT=bhsb[:, k:k + 2, 128:128 + h_dim],
                        rhs=bhsb[:, k:k + 2, 0:128],
                        start=(j == 0), stop=(j == S // 2 - 1),
                        perf_mode=DRM,
                    )
                if relu_w2:
                    nc.scalar.activation(
                        out=uT[:, 128 * t:128 * (t + 1)], in_=acc[:],
                        func=mybir.ActivationFunctionType.Relu,
                    )
                else:
                    eng = nc.vector.tensor_copy if t % 2 == 0 else nc.scalar.activation
                    kw = {} if t % 2 == 0 else {"func": mybir.ActivationFunctionType.Copy}
                    eng(out=osb[:, 128 * t:128 * (t + 1)], in_=acc[:], **kw)
            if relu_w2:
                for i, (n0, nn) in enumerate([(0, 512), (512, 512), (1024, 256)]):
                    ps = psw.tile([H2, nn], mybir.dt.float32, tag=f"w{i}")
                    nc.tensor.matmul(out=ps[:], lhsT=w2sb[:], rhs=uT[:, n0:n0 + nn],
                                     start=True, stop=True)
                    nc.vector.tensor_copy(out=osb[:, n0:n0 + nn], in_=ps[:])
            nc.sync.dma_start(out=out[:], in_=osb[:])
    nc.compile()
    return nc


# l4 geometry: per dest tile t we keep cols [k0, M_SH) of every remote core.
L4_W = [M_SH - 128 * t for t in range(NT)]
L4_OFF = np.concatenate([[0], np.cumsum([NC * w for w in L4_W])]).astype(int)
L4_TOT = int(L4_OFF[-1])


def _build_l4():
    nc = _new_nc()
    ztin = nc.dram_tensor("ztin", [128, NC, M_PAD], mybir.dt.bfloat16, kind="ExternalInput")
    outd = nc.dram_tensor("out", [128, L4_TOT], mybir.dt.bfloat16, kind="ExternalOutput")
    with tile.TileContext(nc) as tc:
        with (
            tc.tile_pool(name="sb", bufs=1) as sb,
            tc.tile_pool(name="stg", bufs=3) as stg,
            tc.tile_pool(name="ps4", bufs=8, space="PSUM") as ps4,
        ):
            ztr = sb.tile([128, NC, M_PAD], mybir.dt.bfloat16)
            # own z (j=0) lands first so the t=0 quads start early
            nc.sync.dma_start(out=ztr[:, 0:2, :], in_=ztin[:, 0:2, :])
            nc.scalar.dma_start(out=ztr[:, 2:NC, :], in_=ztin[:, 2:NC, :])
            eng = 0
            for t in range(NT):
                k0 = 128 * t
                Wt = L4_W[t]
                stage = stg.tile([128, NC * Wt], mybir.dt.bfloat16, tag="stage")
                chunks = []
                for j in range(NC):
                    off = k0
                    while off < M_SH:
                        nn = min(512, M_SH - off)
                        chunks.append((j, off, nn))
                        off += nn
                for q0 in range(0, len(chunks), 4):
                    quad = chunks[q0:q0 + 4]
                    pss4 = []
                    for g, (j, off, nn) in enumerate(quad):
                        ps = ps4.tile([128, 512], mybir.dt.float32, tag="l4")
                        nc.tensor.matmul(
                            out=ps[:, :nn],
                            lhsT=ztr[32 * g:32 * g + H2, 0, k0:k0 + 128],
                            rhs=ztr[32 * g:32 * g + H2, j, off:off + nn],
                            start=True, stop=True,
                            tile_position=(32 * g, 0),
                        )
                        pss4.append(ps)
                    for (j, off, nn), ps in zip(quad, pss4):
                        dst = stage[:, j * Wt + (off - k0):j * Wt + (off - k0) + nn]
                        if eng % 2 == 0:
                            nc.vector.tensor_copy(out=dst, in_=ps[:, :nn])
                        else:
                            nc.scalar.activation(out=dst, in_=ps[:, :nn],
                                                 func=mybir.ActivationFunctionType.Copy)
                        eng += 1
                dq = nc.sync if t % 2 == 0 else nc.scalar
                dq.dma_start(out=outd[:, int(L4_OFF[t]):int(L4_OFF[t + 1])], in_=stage[:])
    nc.compile()
    return nc


def _get(name, builder):
    if name not in _cache:
        _cache[name] = builder()
    return _cache[name]


def _run(nc, in_maps, name=None, check_key=None):
    if name is not None:
        _last_maps[name] = in_maps
        _last_nc[name] = nc
    for _ in range(3):
        res = run_bass_kernel_spmd(nc, in_maps, list(range(NC))).results
        if check_key is None:
            return res
        ok = all(np.abs(np.asarray(res[c][check_key], np.float32)).max() > 0
                 for c in range(NC))
        if ok:
            return res
    return res


def kernel(x, edge_w, W1, W2, edge_row, edge_col):
    x = np.asarray(x, np.float32)
    ew = np.asarray(edge_w, np.float32)
    W1 = np.asarray(W1, np.float32)
    W2 = np.asarray(W2, np.float32)
    er = np.asarray(edge_row).astype(np.int64)
    ec = np.asarray(edge_col).astype(np.int64)

    # ---- per-(core, dest-tile) source compaction (host, uncounted)
    core = er % NC
    loc = er // NC
    tl = loc // 128
    dloc = loc - 128 * tl
    order = np.lexsort((ec, tl, core))
    cs, ts_, es, ds, ws = core[order], tl[order], ec[order], dloc[order], ew[order]
    key = cs * NT + ts_
    uniq, starts = np.unique(key, return_index=True)
    starts = list(starts) + [len(key)]
    max_distinct = 0
    seg = {}
    for i, k in enumerate(uniq):
        sl = slice(starts[i], starts[i + 1])
        src, inv = np.unique(es[sl], return_inverse=True)
        seg[int(k)] = (src, inv, ds[sl], ws[sl])
        max_distinct = max(max_distinct, len(src))
    S = max(2, -(-max_distinct // 256) * 2)  # k-slices of 128, even for DoubleRow
    NMAX = S * 128
    KB = NT * S

    # ---- l1: h1 = x_own @ W1
    w1d = np.ascontiguousarray(W1.reshape(KX, 128, H1).transpose(1, 0, 2)).astype(BF16)
    xts = []
    for c in range(NC):
        xa = np.zeros((F, M_PAD), np.float32)
        xa[:, :M_SH] = x[c::NC, :].T
        xts.append(np.ascontiguousarray(
            xa.reshape(KX, 128, M_PAD).transpose(1, 0, 2)).astype(BF16))
    l1 = _get("l1", _build_l1)
    res = _run(l1, [{"xt": xts[c], "w1": w1d} for c in range(NC)], "l1", check_key="h1t")
    h1 = np.zeros((N, H1), np.float32)
    for c in range(NC):
        h1[c::NC] = res[c]["h1t"].astype(np.float32)[:, :M_SH].T

    # ---- build per-core bh tensors (B tiles shared by l2/l3)
    def build_bh(hfull_q, h_dim):
        Wd = 128 + h_dim
        maps = []
        for c in range(NC):
            bhc = np.zeros((128, KB, Wd), FP8)
            for t in range(NT):
                k = c * NT + t
                if k not in seg:
                    continue
                src, inv, dl, wv = seg[k]
                B = np.zeros((NMAX, 128), np.float32)
                np.add.at(B, (inv, dl), wv)
                B3 = B.reshape(S, 128, 128).transpose(1, 0, 2)
                bhc[:, t * S:(t + 1) * S, 0:128] = B3.astype(FP8)
                hg = np.zeros((NMAX, h_dim), FP8)
                hg[:len(src)] = hfull_q[src]
                bhc[:, t * S:(t + 1) * S, 128:] = hg.reshape(S, 128, h_dim).transpose(1, 0, 2)
            maps.append(bhc)
        return maps

    h1q = h1.astype(FP8)
    l2 = _get(f"l2_{S}", lambda: _build_spmm(S, H1, True))
    bh1 = build_bh(h1q, H1)
    res = _run(l2, [{"bh": bh1[c], "w2": W2} for c in range(NC)], "l2", check_key="tt")
    tfull = np.zeros((N, H2), np.float32)
    for c in range(NC):
        tfull[c::NC] = res[c]["tt"].astype(np.float32)[:, :M_SH].T

    # ---- l3
    tq = tfull.astype(FP8)
    l3 = _get(f"l3_{S}", lambda: _build_spmm(S, H2, False))
    bh2 = build_bh(tq, H2)
    res = _run(l3, [{"bh": bh2[c]} for c in range(NC)], "l3", check_key="zt")
    zf = np.stack([res[c]["zt"] for c in range(NC)], axis=0)  # [NC, H2, M_PAD] bf16

    # ---- l4: z @ z^T triangle; per-core ci rotation puts own z first,
    # replicated into the four 32-partition groups for the quad positions
    maps = []
    for c in range(NC):
        ztin = np.zeros((128, NC, M_PAD), BF16)
        for j in range(NC):
            for g in range(4):
                ztin[32 * g:32 * g + H2, j, :] = zf[(c + j) % NC]
        maps.append({"ztin": ztin})
    l4 = _get("l4", _build_l4)
    res = _run(l4, maps, "l4", check_key="out")

    Fm = np.empty((N, N), np.float32)
    F4 = Fm.reshape(M_SH, NC, M_SH, NC)
    for c in range(NC):
        O = res[c]["out"].astype(np.float32)
        for t in range(NT):
            k0 = 128 * t
            Wt = L4_W[t]
            pr = min(128, M_SH - k0)
            blk = O[:pr, int(L4_OFF[t]):int(L4_OFF[t + 1])].reshape(pr, NC, Wt)
            for j in range(NC):
                cj = (c + j) % NC
                F4[k0:k0 + pr, c, k0:M_SH, cj] = blk[:, j, :]
                F4[k0:M_SH, cj, k0:k0 + pr, c] = blk[:, j, :].T
    return Fm.reshape(-1)


# revision 17
# speedup vs baseline: 1.0038x; 1.0038x over previous
"""GCN autoencoder on 8 trn2 cores — 4 launches, no collectives.

Interleaved dest-row sharding (core c owns global rows g with g%8==c,
local index g//8). Host does the gathers between launches (uncounted).

  l1: own-rows (x @ W1)^T, W1-stationary             -> h1T [32, M_PAD] bf16
  l2: compact SpMM (A @ h1) + relu + @W2             -> tT  [16, M_PAD] bf16
  l3: compact SpMM (A @ t)                           -> zT  [16, M_PAD] bf16
  l4: z @ z^T upper-tri n-tiles, 4-way tile_position quads, contiguous
      per-t bf16 writes; host mirrors the triangle.

The SpMM avoids the dense 10240x10240 A: for each 128-dest-row tile only
~3.3k of the 10240 source nodes have an edge into it, so the host builds a
per-tile compacted B [NMAX,128] (B[s,d] = sum of edge weights s->d) plus the
matching gathered h rows, packed into one fp8 tensor bh [128, NTILE*S, 128+h].
u^T_tile = (h_c)^T @ B via DoubleRow fp8 matmuls.
"""

import sys

sys.path.insert(0, "/opt/trn_rl_repo")

import numpy as np
import ml_dtypes

import concourse.bacc as bacc
import concourse.mybir as mybir
import concourse.tile as tile
from concourse.bass_utils import run_bass_kernel_spmd

BF16 = ml_dtypes.bfloat16
FP8 = ml_dtypes.float8_e4m3

NC = 8
N = 10000
F = 512
H1 = 32
H2 = 16
M_SH = N // NC  # 1250 rows per core
M_PAD = 1280
NT = M_PAD // 128  # 10 dest tiles per core
KX = F // 128

_cache = {}
_last_maps = {}
_last_nc = {}
_launch_order = ["l1", "l2", "l3", "l4"]


def _new_nc():
    return bacc.Bacc("TRN2", target_bir_lowering=False, debug=False, num_devices=NC)


def _build_l1():
    """h1T = (x_own @ W1)^T, W1-slice stationary, x streaming."""
    nc = _new_nc()
    xt = nc.dram_tensor("xt", [128, KX, M_PAD], mybir.dt.bfloat16, kind="ExternalInput")
    w1 = nc.dram_tensor("w1", [128, KX, H1], mybir.dt.bfloat16, kind="ExternalInput")
    out = nc.dram_tensor("h1t", [H1, M_PAD], mybir.dt.bfloat16, kind="ExternalOutput")
    CH = [(0, 512), (512, 512), (1024, 256)]
    with tile.TileContext(nc) as tc:
        with (
            tc.tile_pool(name="sb", bufs=1) as sb,
            tc.tile_pool(name="pss", bufs=1, space="PSUM") as pss,
        ):
            xsb = sb.tile([128, KX, M_PAD], mybir.dt.bfloat16)
            w1sb = sb.tile([128, KX, H1], mybir.dt.bfloat16)
            h1sb = sb.tile([H1, M_PAD], mybir.dt.bfloat16)
            nc.sync.dma_start(out=w1sb[:], in_=w1[:])
            for kx in range(KX):
                dq = nc.scalar if kx % 2 == 0 else nc.sync
                dq.dma_start(out=xsb[:, kx, :], in_=xt[:, kx, :])
            pss_t = [pss.tile([H1, nn], mybir.dt.float32, tag=f"c{i}", name=f"c{i}")
                     for i, (n0, nn) in enumerate(CH)]
            for kx in range(KX):
                for i, (n0, nn) in enumerate(CH):
                    nc.tensor.matmul(
                        out=pss_t[i][:], lhsT=w1sb[:, kx, :],
                        rhs=xsb[:, kx, n0:n0 + nn],
                        start=(kx == 0), stop=(kx == KX - 1),
                    )
            for i, (n0, nn) in enumerate(CH):
                eng = [nc.vector.tensor_copy, nc.scalar.activation, nc.vector.tensor_copy][i]
                kw = {} if i != 1 else {"func": mybir.ActivationFunctionType.Copy}
                eng(out=h1sb[:, n0:n0 + nn], in_=pss_t[i][:], **kw)
            nc.sync.dma_start(out=out[:], in_=h1sb[:])
    nc.compile()
    return nc


def _build_spmm(S, h_dim, relu_w2):
    """Compact A-contraction: per dest tile, S k-slices of 128 compacted
    sources; bh[:, tile*S+s, :128] = B rows, [128:128+h_dim] = gathered h.
    relu_w2: apply relu then @W2 (l2) else plain copy-out (l3)."""
    nc = _new_nc()
    W = 128 + h_dim
    KB = NT * S
    bh = nc.dram_tensor("bh", [128, KB, W], mybir.dt.float8e4, kind="ExternalInput")
    if relu_w2:
        w2 = nc.dram_tensor("w2", [H1, H2], mybir.dt.float32, kind="ExternalInput")
        out = nc.dram_tensor("tt", [H2, M_PAD], mybir.dt.bfloat16, kind="ExternalOutput")
    else:
        out = nc.dram_tensor("zt", [h_dim, M_PAD], mybir.dt.bfloat16, kind="ExternalOutput")
    DRM = mybir.MatmulPerfMode.DoubleRow
    NCHUNK = 10
    with tile.TileContext(nc) as tc:
        with (
            tc.tile_pool(name="sb", bufs=1) as sb,
            tc.tile_pool(name="psu", bufs=4, space="PSUM") as psu,
            tc.tile_pool(name="psw", bufs=1, space="PSUM") as psw,
        ):
            bhsb = sb.tile([128, KB, W], mybir.dt.float8e4)
            per = KB // NCHUNK
            for b in range(NCHUNK):
                dq = nc.sync if b % 2 == 0 else nc.scalar
                dq.dma_start(out=bhsb[:, b * per:(b + 1) * per, :],
                             in_=bh[:, b * per:(b + 1) * per, :])
            if relu_w2:
                w2sb = sb.tile([H1, H2], mybir.dt.float32)
                nc.sync.dma_start(out=w2sb[:], in_=w2[:])
                uT = sb.tile([H1, M_PAD], mybir.dt.float32)
            osb = sb.tile([h_dim if not relu_w2 else H2, M_PAD], mybir.dt.bfloat16)
            for t in range(NT):
                acc = psu.tile([h_dim, 128], mybir.dt.float32, tag="u")
                for j in range(S // 2):
                    k = t * S + 2 * j
                    nc.tensor.matmul(
                        out=acc[:], lhsT=bhsb[:, k:k + 2, 128:128 + h_dim],
                        rhs=bhsb[:, k:k + 2, 0:128],
                        start=(j == 0), stop=(j == S // 2 - 1),
                        perf_mode=DRM,
                    )
                if relu_w2:
                    nc.scalar.activation(
                        out=uT[:, 128 * t:128 * (t + 1)], in_=acc[:],
                        func=mybir.ActivationFunctionType.Relu,
                    )
                else:
                    eng = nc.vector.tensor_copy if t % 2 == 0 else nc.scalar.activation
                    kw = {} if t % 2 == 0 else {"func": mybir.ActivationFunctionType.Copy}
                    eng(out=osb[:, 128 * t:128 * (t + 1)], in_=acc[:], **kw)
            if relu_w2:
                for i, (n0, nn) in enumerate([(0, 512), (512, 512), (1024, 256)]):
                    ps = psw.tile([H2, nn], mybir.dt.float32, tag=f"w{i}")
                    nc.tensor.matmul(out=ps[:], lhsT=w2sb[:], rhs=uT[:, n0:n0 + nn],
                                     start=True, stop=True)
                    nc.vector.tensor_copy(out=osb[:, n0:n0 + nn], in_=ps[:])
            nc.sync.dma_start(out=out[:], in_=osb[:])
    nc.compile()
    return nc


# l4 geometry: per dest tile t we keep cols [k0, M_SH) of every remote core.
L4_W = [M_SH - 128 * t for t in range(NT)]
L4_OFF = np.concatenate([[0], np.cumsum([NC * w for w in L4_W])]).astype(int)
L4_TOT = int(L4_OFF[-1])


def _build_l4():
    nc = _new_nc()
    ztin = nc.dram_tensor("ztin", [128, NC, M_PAD], mybir.dt.bfloat16, kind="ExternalInput")
    outd = nc.dram_tensor("out", [128, L4_TOT], mybir.dt.bfloat16, kind="ExternalOutput")
    with tile.TileContext(nc) as tc:
        with (
            tc.tile_pool(name="sb", bufs=1) as sb,
            tc.tile_pool(name="stg", bufs=3) as stg,
            tc.tile_pool(name="ps4", bufs=2, space="PSUM") as ps4,
        ):
            ztr = sb.tile([128, NC, M_PAD], mybir.dt.bfloat16)
            # own z (j=0) lands first so the t=0 quads start early
            nc.sync.dma_start(out=ztr[:, 0:2, :], in_=ztin[:, 0:2, :])
            nc.scalar.dma_start(out=ztr[:, 2:5, :], in_=ztin[:, 2:5, :])
            nc.sync.dma_start(out=ztr[:, 5:NC, :], in_=ztin[:, 5:NC, :])
            eng = 0
            for t in range(NT):
                k0 = 128 * t
                Wt = L4_W[t]
                stage = stg.tile([128, NC, Wt], mybir.dt.bfloat16, tag="stage")
                offs = []
                off = k0
                while off < M_SH:
                    offs.append((off, min(512, M_SH - off)))
                    off += 512
                for off, nn in offs:
                    for jh in range(2):
                        ps = ps4.tile([128, 4, 512], mybir.dt.float32, tag="l4")
                        for q in range(4):
                            j = 4 * jh + q
                            nc.tensor.matmul(
                                out=ps[:, q, :nn],
                                lhsT=ztr[32 * q:32 * q + H2, 0, k0:k0 + 128],
                                rhs=ztr[32 * q:32 * q + H2, j, off:off + nn],
                                start=True, stop=True,
                                tile_position=(32 * q, 0),
                            )
                        dst = stage[:, 4 * jh:4 * jh + 4, off - k0:off - k0 + nn]
                        if eng % 2 == 0:
                            nc.vector.tensor_copy(out=dst, in_=ps[:, 0:4, 0:nn])
                        else:
                            nc.scalar.activation(out=dst, in_=ps[:, 0:4, 0:nn],
                                                 func=mybir.ActivationFunctionType.Copy)
                        eng += 1
                dq = nc.sync if t % 2 == 0 else nc.scalar
                dq.dma_start(out=outd[:, int(L4_OFF[t]):int(L4_OFF[t + 1])], in_=stage[:])
    nc.compile()
    return nc


def _get(name, builder):
    if name not in _cache:
        _cache[name] = builder()
    return _cache[name]


def _run(nc, in_maps, name=None, check_key=None):
    if name is not None:
        _last_maps[name] = in_maps
        _last_nc[name] = nc
    for _ in range(3):
        res = run_bass_kernel_spmd(nc, in_maps, list(range(NC))).results
        if check_key is None:
            return res
        ok = all(np.abs(np.asarray(res[c][check_key], np.float32)).max() > 0
                 for c in range(NC))
        if ok:
            return res
    return res


def kernel(x, edge_w, W1, W2, edge_row, edge_col):
    x = np.asarray(x, np.float32)
    ew = np.asarray(edge_w, np.float32)
    W1 = np.asarray(W1, np.float32)
    W2 = np.asarray(W2, np.float32)
    er = np.asarray(edge_row).astype(np.int64)
    ec = np.asarray(edge_col).astype(np.int64)

    # ---- per-(core, dest-tile) source compaction (host, uncounted)
    core = er % NC
    loc = er // NC
    tl = loc // 128
    dloc = loc - 128 * tl
    order = np.lexsort((ec, tl, core))
    cs, ts_, es, ds, ws = core[order], tl[order], ec[order], dloc[order], ew[order]
    key = cs * NT + ts_
    uniq, starts = np.unique(key, return_index=True)
    starts = list(starts) + [len(key)]
    max_distinct = 0
    seg = {}
    for i, k in enumerate(uniq):
        sl = slice(starts[i], starts[i + 1])
        src, inv = np.unique(es[sl], return_inverse=True)
        seg[int(k)] = (src, inv, ds[sl], ws[sl])
        max_distinct = max(max_distinct, len(src))
    S = max(2, -(-max_distinct // 256) * 2)  # k-slices of 128, even for DoubleRow
    NMAX = S * 128
    KB = NT * S

    # ---- l1: h1 = x_own @ W1
    w1d = np.ascontiguousarray(W1.reshape(KX, 128, H1).transpose(1, 0, 2)).astype(BF16)
    xts = []
    for c in range(NC):
        xa = np.zeros((F, M_PAD), np.float32)
        xa[:, :M_SH] = x[c::NC, :].T
        xts.append(np.ascontiguousarray(
            xa.reshape(KX, 128, M_PAD).transpose(1, 0, 2)).astype(BF16))
    l1 = _get("l1", _build_l1)
    res = _run(l1, [{"xt": xts[c], "w1": w1d} for c in range(NC)], "l1", check_key="h1t")
    h1 = np.zeros((N, H1), np.float32)
    for c in range(NC):
        h1[c::NC] = res[c]["h1t"].astype(np.float32)[:, :M_SH].T

    # ---- build per-core bh tensors (B tiles shared by l2/l3)
    def build_bh(hfull_q, h_dim):
        Wd = 128 + h_dim
        maps = []
        for c in range(NC):
            bhc = np.zeros((128, KB, Wd), FP8)
            for t in range(NT):
                k = c * NT + t
                if k not in seg:
                    continue
                src, inv, dl, wv = seg[k]
                B = np.zeros((NMAX, 128), np.float32)
                np.add.at(B, (inv, dl), wv)
                B3 = B.reshape(S, 128, 128).transpose(1, 0, 2)
                bhc[:, t * S:(t + 1) * S, 0:128] = B3.astype(FP8)
                hg = np.zeros((NMAX, h_dim), FP8)
                hg[:len(src)] = hfull_q[src]
                bhc[:, t * S:(t + 1) * S, 128:] = hg.reshape(S, 128, h_dim).transpose(1, 0, 2)
            maps.append(bhc)
        return maps

    h1q = h1.astype(FP8)
    l2 = _get(f"l2_{S}", lambda: _build_spmm(S, H1, True))
    bh1 = build_bh(h1q, H1)
    res = _run(l2, [{"bh": bh1[c], "w2": W2} for c in range(NC)], "l2", check_key="tt")
    tfull = np.zeros((N, H2), np.float32)
    for c in range(NC):
        tfull[c::NC] = res[c]["tt"].astype(np.float32)[:, :M_SH].T

    # ---- l3
    tq = tfull.astype(FP8)
    l3 = _get(f"l3_{S}", lambda: _build_spmm(S, H2, False))
    bh2 = build_bh(tq, H2)
    res = _run(l3, [{"bh": bh2[c]} for c in range(NC)], "l3", check_key="zt")
    zf = np.stack([res[c]["zt"] for c in range(NC)], axis=0)  # [NC, H2, M_PAD] bf16

    # ---- l4: z @ z^T triangle; per-core ci rotation puts own z first,
    # replicated into the four 32-partition groups for the quad positions
    maps = []
    for c in range(NC):
        ztin = np.zeros((128, NC, M_PAD), BF16)
        for j in range(NC):
            for g in range(4):
                ztin[32 * g:32 * g + H2, j, :] = zf[(c + j) % NC]
        maps.append({"ztin": ztin})
    l4 = _get("l4", _build_l4)
    res = _run(l4, maps, "l4", check_key="out")

    Fm = np.empty((N, N), np.float32)
    F4 = Fm.reshape(M_SH, NC, M_SH, NC)
    for c in range(NC):
        O = res[c]["out"].astype(np.float32)
        for t in range(NT):
            k0 = 128 * t
            Wt = L4_W[t]
            pr = min(128, M_SH - k0)
            blk = O[:pr, int(L4_OFF[t]):int(L4_OFF[t + 1])].reshape(pr, NC, Wt)
            for j in range(NC):
                cj = (c + j) % NC
                F4[k0:k0 + pr, c, k0:M_SH, cj] = blk[:, j, :]
                F4[k0:M_SH, cj, k0:k0 + pr, c] = blk[:, j, :].T
    return Fm.reshape(-1)


# revision 24
# speedup vs baseline: 1.0926x; 1.0885x over previous
"""GCN autoencoder on 8 trn2 cores — 4 launches, no collectives.

Interleaved dest-row sharding (core c owns global rows g with g%8==c,
local index g//8). Host does the gathers between launches (uncounted).

  l1: own-rows (x @ W1)^T, W1-stationary             -> h1T [32, M_PAD] bf16
  l2: compact SpMM (A @ h1) + relu + @W2             -> tT  [16, M_PAD] bf16
  l3: compact SpMM (A @ t)                           -> zT  [16, M_PAD] bf16
  l4: z @ z^T upper-tri n-tiles, 4-way tile_position quads, contiguous
      per-t bf16 writes; host mirrors the triangle.

The SpMM avoids the dense 10240x10240 A: for each 128-dest-row tile only
~3.3k of the 10240 source nodes have an edge into it, so the host builds a
per-tile compacted B [NMAX,128] (B[s,d] = sum of edge weights s->d) plus the
matching gathered h rows, packed into one fp8 tensor bh [128, NTILE*S, 128+h].
u^T_tile = (h_c)^T @ B via DoubleRow fp8 matmuls.
"""

import sys

sys.path.insert(0, "/opt/trn_rl_repo")

import numpy as np
import ml_dtypes

import concourse.bacc as bacc
import concourse.mybir as mybir
import concourse.tile as tile
from concourse.bass_utils import run_bass_kernel_spmd

BF16 = ml_dtypes.bfloat16
FP8 = ml_dtypes.float8_e4m3

NC = 8
N = 10000
F = 512
H1 = 32
H2 = 16
M_SH = N // NC  # 1250 rows per core
M_PAD = 1280
NT = M_PAD // 128  # 10 dest tiles per core
KX = F // 128

_cache = {}
_last_maps = {}
_last_nc = {}
_launch_order = ["l1", "l2", "l3", "l4"]
_WRM = np.zeros((128, 512), BF16)


def _new_nc():
    return bacc.Bacc("TRN2", target_bir_lowering=False, debug=False, num_devices=NC)


def _warmup(nc, sb, pool, wrm, n=8):
    """Dummy matmuls right after the preamble keep the PE busy so the HAM
    clock gate releases (1.2 -> 2.4 GHz) before the real matmuls start."""
    wrmsb = sb.tile([128, 512], mybir.dt.bfloat16, name="wrmsb")
    nc.sync.dma_start(out=wrmsb[:], in_=wrm[:])
    pswu = pool.tile([128, 512], mybir.dt.float32, tag="wu", name="pswu")
    for _ in range(n):
        nc.tensor.matmul(out=pswu[:], lhsT=wrmsb[:, 0:128], rhs=wrmsb[:],
                         start=True, stop=True)


def _build_l1():
    """h1T = (x_own @ W1)^T, W1-slice stationary, x streaming."""
    nc = _new_nc()
    xt = nc.dram_tensor("xt", [128, KX, M_PAD], mybir.dt.bfloat16, kind="ExternalInput")
    w1 = nc.dram_tensor("w1", [128, KX, H1], mybir.dt.bfloat16, kind="ExternalInput")
    wrm = nc.dram_tensor("wrm", [128, 512], mybir.dt.bfloat16, kind="ExternalInput")
    out = nc.dram_tensor("h1t", [H1, M_PAD], mybir.dt.bfloat16, kind="ExternalOutput")
    CH = [(0, 512), (512, 512), (1024, 256)]
    with tile.TileContext(nc) as tc:
        with (
            tc.tile_pool(name="sb", bufs=1) as sb,
            tc.tile_pool(name="pss", bufs=1, space="PSUM") as pss,
        ):
            xsb = sb.tile([128, KX, M_PAD], mybir.dt.bfloat16)
            w1sb = sb.tile([128, KX, H1], mybir.dt.bfloat16)
            h1sb = sb.tile([H1, M_PAD], mybir.dt.bfloat16)
            _warmup(nc, sb, pss, wrm)
            nc.sync.dma_start(out=w1sb[:], in_=w1[:])
            for kx in range(KX):
                dq = nc.scalar if kx % 2 == 0 else nc.sync
                dq.dma_start(out=xsb[:, kx, :], in_=xt[:, kx, :])
            pss_t = [pss.tile([H1, nn], mybir.dt.float32, tag=f"c{i}", name=f"c{i}")
                     for i, (n0, nn) in enumerate(CH)]
            for kx in range(KX):
                for i, (n0, nn) in enumerate(CH):
                    nc.tensor.matmul(
                        out=pss_t[i][:], lhsT=w1sb[:, kx, :],
                        rhs=xsb[:, kx, n0:n0 + nn],
                        start=(kx == 0), stop=(kx == KX - 1),
                    )
            for i, (n0, nn) in enumerate(CH):
                eng = [nc.vector.tensor_copy, nc.scalar.activation, nc.vector.tensor_copy][i]
                kw = {} if i != 1 else {"func": mybir.ActivationFunctionType.Copy}
                eng(out=h1sb[:, n0:n0 + nn], in_=pss_t[i][:], **kw)
            nc.sync.dma_start(out=out[:], in_=h1sb[:])
    nc.compile()
    return nc


def _build_spmm(S, h_dim, relu_w2):
    """Compact A-contraction: per dest tile, S k-slices of 128 compacted
    sources; bh[:, tile*S+s, :128] = B rows, [128:128+h_dim] = gathered h.
    relu_w2: apply relu then @W2 (l2) else plain copy-out (l3)."""
    nc = _new_nc()
    W = 128 + h_dim
    KB = NT * S
    bh = nc.dram_tensor("bh", [128, KB, W], mybir.dt.float8e4, kind="ExternalInput")
    wrm = nc.dram_tensor("wrm", [128, 512], mybir.dt.bfloat16, kind="ExternalInput")
    if relu_w2:
        w2 = nc.dram_tensor("w2", [H1, H2], mybir.dt.float32, kind="ExternalInput")
        out = nc.dram_tensor("tt", [H2, M_PAD], mybir.dt.bfloat16, kind="ExternalOutput")
    else:
        out = nc.dram_tensor("zt", [h_dim, M_PAD], mybir.dt.bfloat16, kind="ExternalOutput")
    DRM = mybir.MatmulPerfMode.DoubleRow
    NCHUNK = 5
    with tile.TileContext(nc) as tc:
        with (
            tc.tile_pool(name="sb", bufs=1) as sb,
            tc.tile_pool(name="psu", bufs=4, space="PSUM") as psu,
            tc.tile_pool(name="psw", bufs=1, space="PSUM") as psw,
        ):
            bhsb = sb.tile([128, KB, W], mybir.dt.float8e4)
            _warmup(nc, sb, psw, wrm)
            per = KB // NCHUNK
            for b in range(NCHUNK):
                dq = nc.sync if b % 2 == 0 else nc.scalar
                dq.dma_start(out=bhsb[:, b * per:(b + 1) * per, :],
                             in_=bh[:, b * per:(b + 1) * per, :])
            if relu_w2:
                w2sb = sb.tile([H1, H2], mybir.dt.float32)
                nc.sync.dma_start(out=w2sb[:], in_=w2[:])
                uT = sb.tile([H1, M_PAD], mybir.dt.float32)
            osb = sb.tile([h_dim if not relu_w2 else H2, M_PAD], mybir.dt.bfloat16)
            for t in range(NT):
                acc = psu.tile([h_dim, 128], mybir.dt.float32, tag="u")
                for j in range(S // 2):
                    k = t * S + 2 * j
                    nc.tensor.matmul(
                        out=acc[:], lhsT=bhsb[:, k:k + 2, 128:128 + h_dim],
                        rhs=bhsb[:, k:k + 2, 0:128],
                        start=(j == 0), stop=(j == S // 2 - 1),
                        perf_mode=DRM,
                    )
                if relu_w2:
                    nc.scalar.activation(
                        out=uT[:, 128 * t:128 * (t + 1)], in_=acc[:],
                        func=mybir.ActivationFunctionType.Relu,
                    )
                else:
                    eng = nc.vector.tensor_copy if t % 2 == 0 else nc.scalar.activation
                    kw = {} if t % 2 == 0 else {"func": mybir.ActivationFunctionType.Copy}
                    eng(out=osb[:, 128 * t:128 * (t + 1)], in_=acc[:], **kw)
            if relu_w2:
                for i, (n0, nn) in enumerate([(0, 512), (512, 512), (1024, 256)]):
                    ps = psw.tile([H2, nn], mybir.dt.float32, tag=f"w{i}")
                    nc.tensor.matmul(out=ps[:], lhsT=w2sb[:], rhs=uT[:, n0:n0 + nn],
                                     start=True, stop=True)
                    nc.vector.tensor_copy(out=osb[:, n0:n0 + nn], in_=ps[:])
            nc.sync.dma_start(out=out[:], in_=osb[:])
    nc.compile()
    return nc


# l4 geometry: per dest tile t we keep cols [k0, M_SH) of every remote core.
L4_W = [M_SH - 128 * t for t in range(NT)]
L4_OFF = np.concatenate([[0], np.cumsum([NC * w for w in L4_W])]).astype(int)
L4_TOT = int(L4_OFF[-1])


def _build_l4():
    nc = _new_nc()
    ztin = nc.dram_tensor("ztin", [128, NC, M_PAD], mybir.dt.bfloat16, kind="ExternalInput")
    wrm = nc.dram_tensor("wrm", [128, 512], mybir.dt.bfloat16, kind="ExternalInput")
    outd = nc.dram_tensor("out", [128, L4_TOT], mybir.dt.bfloat16, kind="ExternalOutput")
    with tile.TileContext(nc) as tc:
        with (
            tc.tile_pool(name="sb", bufs=1) as sb,
            tc.tile_pool(name="stg", bufs=3) as stg,
            tc.tile_pool(name="ps4", bufs=2, space="PSUM") as ps4,
        ):
            ztr = sb.tile([128, NC, M_PAD], mybir.dt.bfloat16)
            wrmsb = sb.tile([128, 512], mybir.dt.bfloat16, name="wrmsb")
            nc.sync.dma_start(out=wrmsb[:], in_=wrm[:])
            wps = ps4.tile([128, 4, 512], mybir.dt.float32, tag="l4", name="wps")
            for _ in range(8):
                nc.tensor.matmul(out=wps[:, 0, :], lhsT=wrmsb[:, 0:128], rhs=wrmsb[:],
                                 start=True, stop=True)
            # own z (j=0) lands first so the t=0 quads start early
            nc.sync.dma_start(out=ztr[:, 0:2, :], in_=ztin[:, 0:2, :])
            nc.scalar.dma_start(out=ztr[:, 2:5, :], in_=ztin[:, 2:5, :])
            nc.sync.dma_start(out=ztr[:, 5:NC, :], in_=ztin[:, 5:NC, :])
            for t in range(NT):
                k0 = 128 * t
                Wt = L4_W[t]
                stage = stg.tile([128, NC, Wt], mybir.dt.bfloat16, tag="stage")
                offs = []
                off = k0
                while off < M_SH:
                    offs.append((off, min(512, M_SH - off)))
                    off += 512
                for off, nn in offs:
                    for jh in range(2):
                        ps = ps4.tile([128, 4, 512], mybir.dt.float32, tag="l4")
                        for q in range(4):
                            j = 4 * jh + q
                            nc.tensor.matmul(
                                out=ps[:, q, :nn],
                                lhsT=ztr[32 * q:32 * q + H2, 0, k0:k0 + 128],
                                rhs=ztr[32 * q:32 * q + H2, j, off:off + nn],
                                start=True, stop=True,
                                tile_position=(32 * q, 0),
                            )
                        # split the drain: DVE takes banks 0-1, ACT banks 2-3,
                        # concurrently — halves the PSUM ping-pong latency
                        nc.vector.tensor_copy(
                            out=stage[:, 4 * jh:4 * jh + 2, off - k0:off - k0 + nn],
                            in_=ps[:, 0:2, 0:nn])
                        nc.scalar.activation(
                            out=stage[:, 4 * jh + 2:4 * jh + 4, off - k0:off - k0 + nn],
                            in_=ps[:, 2:4, 0:nn],
                            func=mybir.ActivationFunctionType.Copy)
                dq = nc.sync if t % 2 == 0 else nc.scalar
                dq.dma_start(out=outd[:, int(L4_OFF[t]):int(L4_OFF[t + 1])], in_=stage[:])
    nc.compile()
    return nc


def _get(name, builder):
    if name not in _cache:
        _cache[name] = builder()
    return _cache[name]


def _run(nc, in_maps, name=None, check_key=None):
    if name is not None:
        _last_maps[name] = in_maps
        _last_nc[name] = nc
    for _ in range(3):
        res = run_bass_kernel_spmd(nc, in_maps, list(range(NC))).results
        if check_key is None:
            return res
        ok = all(np.abs(np.asarray(res[c][check_key], np.float32)).max() > 0
                 for c in range(NC))
        if ok:
            return res
    return res


def kernel(x, edge_w, W1, W2, edge_row, edge_col):
    x = np.asarray(x, np.float32)
    ew = np.asarray(edge_w, np.float32)
    W1 = np.asarray(W1, np.float32)
    W2 = np.asarray(W2, np.float32)
    er = np.asarray(edge_row).astype(np.int64)
    ec = np.asarray(edge_col).astype(np.int64)

    # ---- per-(core, dest-tile) source compaction (host, uncounted)
    core = er % NC
    loc = er // NC
    tl = loc // 128
    dloc = loc - 128 * tl
    order = np.lexsort((ec, tl, core))
    cs, ts_, es, ds, ws = core[order], tl[order], ec[order], dloc[order], ew[order]
    key = cs * NT + ts_
    uniq, starts = np.unique(key, return_index=True)
    starts = list(starts) + [len(key)]
    max_distinct = 0
    seg = {}
    for i, k in enumerate(uniq):
        sl = slice(starts[i], starts[i + 1])
        src, inv = np.unique(es[sl], return_inverse=True)
        seg[int(k)] = (src, inv, ds[sl], ws[sl])
        max_distinct = max(max_distinct, len(src))
    S = max(2, -(-max_distinct // 256) * 2)  # k-slices of 128, even for DoubleRow
    NMAX = S * 128
    KB = NT * S

    # ---- l1: h1 = x_own @ W1
    w1d = np.ascontiguousarray(W1.reshape(KX, 128, H1).transpose(1, 0, 2)).astype(BF16)
    xts = []
    for c in range(NC):
        xa = np.zeros((F, M_PAD), np.float32)
        xa[:, :M_SH] = x[c::NC, :].T
        xts.append(np.ascontiguousarray(
            xa.reshape(KX, 128, M_PAD).transpose(1, 0, 2)).astype(BF16))
    l1 = _get("l1", _build_l1)
    res = _run(l1, [{"xt": xts[c], "w1": w1d, "wrm": _WRM} for c in range(NC)], "l1", check_key="h1t")
    h1 = np.zeros((N, H1), np.float32)
    for c in range(NC):
        h1[c::NC] = res[c]["h1t"].astype(np.float32)[:, :M_SH].T

    # ---- build per-core bh tensors (B tiles shared by l2/l3)
    def build_bh(hfull_q, h_dim):
        Wd = 128 + h_dim
        maps = []
        for c in range(NC):
            bhc = np.zeros((128, KB, Wd), FP8)
            for t in range(NT):
                k = c * NT + t
                if k not in seg:
                    continue
                src, inv, dl, wv = seg[k]
                B = np.zeros((NMAX, 128), np.float32)
                np.add.at(B, (inv, dl), wv)
                B3 = B.reshape(S, 128, 128).transpose(1, 0, 2)
                bhc[:, t * S:(t + 1) * S, 0:128] = B3.astype(FP8)
                hg = np.zeros((NMAX, h_dim), FP8)
                hg[:len(src)] = hfull_q[src]
                bhc[:, t * S:(t + 1) * S, 128:] = hg.reshape(S, 128, h_dim).transpose(1, 0, 2)
            maps.append(bhc)
        return maps

    h1q = h1.astype(FP8)
    l2 = _get(f"l2_{S}", lambda: _build_spmm(S, H1, True))
    bh1 = build_bh(h1q, H1)
    res = _run(l2, [{"bh": bh1[c], "w2": W2, "wrm": _WRM} for c in range(NC)], "l2", check_key="tt")
    tfull = np.zeros((N, H2), np.float32)
    for c in range(NC):
        tfull[c::NC] = res[c]["tt"].astype(np.float32)[:, :M_SH].T

    # ---- l3
    tq = tfull.astype(FP8)
    l3 = _get(f"l3_{S}", lambda: _build_spmm(S, H2, False))
    bh2 = build_bh(tq, H2)
    res = _run(l3, [{"bh": bh2[c], "wrm": _WRM} for c in range(NC)], "l3", check_key="zt")
    zf = np.stack([res[c]["zt"] for c in range(NC)], axis=0)  # [NC, H2, M_PAD] bf16

    # ---- l4: z @ z^T triangle; per-core ci rotation puts own z first,
    # replicated into the four 32-partition groups for the quad positions
    maps = []
    for c in range(NC):
        ztin = np.zeros((128, NC, M_PAD), BF16)
        for j in range(NC):
            for g in range(4):
                ztin[32 * g:32 * g + H2, j, :] = zf[(c + j) % NC]
        maps.append({"ztin": ztin, "wrm": _WRM})
    l4 = _get("l4", _build_l4)
    res = _run(l4, maps, "l4", check_key="out")

    Fm = np.empty((N, N), np.float32)
    F4 = Fm.reshape(M_SH, NC, M_SH, NC)
    for c in range(NC):
        O = res[c]["out"].astype(np.float32)
        for t in range(NT):
            k0 = 128 * t
            Wt = L4_W[t]
            pr = min(128, M_SH - k0)
            blk = O[:pr, int(L4_OFF[t]):int(L4_OFF[t + 1])].reshape(pr, NC, Wt)
            for j in range(NC):
                cj = (c + j) % NC
                F4[k0:k0 + pr, c, k0:M_SH, cj] = blk[:, j, :]
                F4[k0:M_SH, cj, k0:k0 + pr, c] = blk[:, j, :].T
    return Fm.reshape(-1)
